# revision 3
# baseline (speedup 1.0000x reference)
"""Deep & Cross Network kernel for 8x Trainium2 NeuronCores (Bass/Tile).

Sharding: pure data-parallel over batch (512 rows/core); weights replicated
per core; no collectives.

Host-side prep inside kernel() (data movement / layout / dtype casts only):
  - embedding lookup x0 = emb[ids] (pure gather), scale by 64, cast fp8e4m3
  - pre-tile weights (scaled by 64, fp8) into SBUF-native DoubleRow layouts

Device math (per core, batch n=512, D=1664), all matmuls fp8 DoubleRow
(0.5 cycles/row, 256-deep contraction per instruction):
  - Deep tower fully transposed: H_l = relu(PSUM/64) with PSUM = (64x)(64w)
    so every intermediate stays in fp8-friendly range. D padded to 14 k-tiles
    (zero plane) so L1 is pure DoubleRow.
  - a_j = x0 . w4_j (w4 = [cross_w_i, out_w[:D]]) computed with examples on
    PARTITIONS by using the x0T slice as the stationary operand -> at [128,16]
  - head lg = ow2 . h3 likewise -> lg_ps [128, 4]
  - cross net collapses algebraically: with u_j = 1 + a_j,
      t3 = (u0 u1 + c1) u2 + c2,  logit = t3*a3 + C + lg + out_b
    (c1, c2, C are cross_b-derived scalars; zero when cross_b == 0)
  - recurrence runs on [128, 4] tiles (examples on partitions).
Early tiny "pacer" matmuls keep the PE p-state ramp alive so real matmuls
run at full clock from the start.
"""

import sys
import os
import numpy as np

for _p in ("/opt/trn_rl_repo",):
    if _p not in sys.path:
        sys.path.insert(0, _p)

import concourse.bass as bass
import concourse.tile as tile
from concourse import bacc, mybir
from concourse import bass_utils

F32 = mybir.dt.float32
FP8 = mybir.dt.float8e4
AF = mybir.ActivationFunctionType
ALU = mybir.AluOpType
AX = mybir.AxisListType
DR = mybir.MatmulPerfMode.DoubleRow

B, F, E, H = 4096, 26, 64, 1000000
D = F * E            # 1664
NC = 8
BC = B // NC         # 512 rows per core
KT = D // 128        # 13 k-tiles over D
KTP = KT + 1         # padded to 14 (zero plane) -> 7 DoubleRow pairs
KP1 = KTP // 2
H1, H2, H3 = 1024, 512, 256
M1, M2, M3 = H1 // 128, H2 // 128, H3 // 128
K2T, K3T, KHT = H1 // 128, H2 // 128, H3 // 128
S = 64.0             # fp8 range scale
INV = float(1.0 / S)
INV2 = float(1.0 / (S * S))

_CACHE = {}


def _emit(tc, zb):
    nc = tc.nc
    a = _CACHE["aps"]
    R = int(os.environ.get("K_REPEAT", "1"))
    L2CONV = os.environ.get("K_L2CONV", "alt")
    DMAV = os.environ.get("K_DMAV", "q2kv")
    L1REORD = os.environ.get("K_L1REORD", "0") == "1"
    MORDER = [6, 7, 0, 1, 2, 3, 4, 5]

    with (
        tc.tile_pool(name="const", bufs=1) as cpool,
        tc.tile_pool(name="act", bufs=1) as apool,
        tc.tile_pool(name="psmm", bufs=6, space="PSUM") as psmm,
        tc.tile_pool(name="psat", bufs=1, space="PSUM") as psat,
        tc.tile_pool(name="pslg", bufs=1, space="PSUM") as pslg,
    ):
        def _body():
            # ---------------- DMA streams (order = priority) --------------
            # Three DMA queues in parallel (each has ~1.28us start cadence):
            # SP: w1/w2 in ~1274ns paired chunks; gpsimd (SWDGE): x0 stream;
            # ACT: tiny/slack transfers.
            w1_sb = cpool.tile([128, M1, KTP, 128], FP8, tag="w1")
            x0_sb = cpool.tile([128, KTP, BC], FP8, tag="x0")
            w2_sb = cpool.tile([128, M2, K2T, 128], FP8, tag="w2")
            w3_sb = cpool.tile([128, M3, K3T, 128], FP8, tag="w3")
            ws_sb = cpool.tile([128, 15, 4], FP8, tag="ws")
            if DMAV == "q3":
                nc.sync.dma_start(w1_sb[:, 0:2], a["w1x"][:, 0:2])
                nc.sync.dma_start(w1_sb[:, 2:4], a["w1x"][:, 2:4])
                nc.sync.dma_start(w1_sb[:, 4:6], a["w1x"][:, 4:6])
                nc.sync.dma_start(w1_sb[:, 6:8], a["w1x"][:, 6:8])
                nc.sync.dma_start(w2_sb[:], a["w2x"][:])
                nc.sync.dma_start(w3_sb[:], a["w3x"][:])
                nc.gpsimd.dma_start(x0_sb[:, 0:8], a["x0T"][:, 0:8])
                nc.gpsimd.dma_start(x0_sb[:, 8:13], a["x0T"][:, 8:13])
                nc.scalar.dma_start(ws_sb[:], a["wsx"][:])
            elif DMAV == "q3b":
                worder = MORDER if L1REORD else [0, 1, 2, 3, 4, 5, 6, 7]
                for i in range(0, 8, 2):
                    lo = worder[i]
                    nc.sync.dma_start(w1_sb[:, lo:lo + 2], a["w1x"][:, lo:lo + 2])
                    if i == 0:
                        nc.sync.dma_start(x0_sb[:, 8:13], a["x0T"][:, 8:13])
                nc.sync.dma_start(w2_sb[:], a["w2x"][:])
                nc.sync.dma_start(w3_sb[:], a["w3x"][:])
                nc.gpsimd.dma_start(x0_sb[:, 0:8], a["x0T"][:, 0:8])
                nc.scalar.dma_start(ws_sb[:], a["wsx"][:])
            elif DMAV == "v3":
                nc.scalar.dma_start(ws_sb[:], a["wsx"][:])
                nc.scalar.dma_start(x0_sb[:, 12:13], a["x0T"][:, 12:13])
                nc.sync.dma_start(w1_sb[:, 0], a["w1x"][:, 0])
                nc.sync.dma_start(x0_sb[:, 0:4], a["x0T"][:, 0:4])
                nc.sync.dma_start(x0_sb[:, 4:8], a["x0T"][:, 4:8])
                nc.sync.dma_start(x0_sb[:, 8:12], a["x0T"][:, 8:12])
                nc.sync.dma_start(w1_sb[:, 1:3], a["w1x"][:, 1:3])
                nc.sync.dma_start(w1_sb[:, 3:5], a["w1x"][:, 3:5])
                nc.sync.dma_start(w1_sb[:, 5:7], a["w1x"][:, 5:7])
                nc.sync.dma_start(w1_sb[:, 7:8], a["w1x"][:, 7:8])
                nc.sync.dma_start(w2_sb[:, 0:2], a["w2x"][:, 0:2])
                nc.sync.dma_start(w2_sb[:, 2:4], a["w2x"][:, 2:4])
                nc.sync.dma_start(w3_sb[:], a["w3x"][:])
            elif DMAV == "q2kv":
                nc.sync.dma_start(w1_sb[:, 0:2], a["w1x"][:, 0:2])
                nc.sync.dma_start(x0_sb[:, 8:13], a["x0T"][:, 8:13])
                nc.sync.dma_start(w1_sb[:, 2:4], a["w1x"][:, 2:4])
                nc.sync.dma_start(w1_sb[:, 4:6], a["w1x"][:, 4:6])
                nc.sync.dma_start(w1_sb[:, 6:8], a["w1x"][:, 6:8])
                nc.sync.dma_start(w2_sb[:], a["w2x"][:])
                nc.sync.dma_start(w3_sb[:], a["w3x"][:])
                nc.scalar.dma_start(x0_sb[:, 0:8], a["x0T"][:, 0:8])
                nc.scalar.dma_start(ws_sb[:], a["wsx"][:])
            elif DMAV == "q3j":
                nc.sync.dma_start(w1_sb[:, 0:2], a["w1x"][:, 0:2])
                nc.sync.dma_start(x0_sb[:, 8:13], a["x0T"][:, 8:13])
                nc.sync.dma_start(w1_sb[:, 2:4], a["w1x"][:, 2:4])
                nc.sync.dma_start(w1_sb[:, 4:6], a["w1x"][:, 4:6])
                nc.sync.dma_start(w1_sb[:, 6:8], a["w1x"][:, 6:8])
                nc.sync.dma_start(w2_sb[:], a["w2x"][:])
                nc.sync.dma_start(w3_sb[:], a["w3x"][:])
                nc.gpsimd.dma_start(x0_sb[:, 2:8], a["x0T"][:, 2:8])
                nc.scalar.dma_start(x0_sb[:, 0:2], a["x0T"][:, 0:2])
                nc.scalar.dma_start(ws_sb[:], a["wsx"][:])
            elif DMAV == "q3k":
                nc.sync.dma_start(w1_sb[:, 0:2], a["w1x"][:, 0:2])
                nc.sync.dma_start(x0_sb[:, 8:13], a["x0T"][:, 8:13])
                nc.sync.dma_start(w1_sb[:, 2:4], a["w1x"][:, 2:4])
                nc.sync.dma_start(w1_sb[:, 4:6], a["w1x"][:, 4:6])
                nc.sync.dma_start(w1_sb[:, 6:8], a["w1x"][:, 6:8])
                nc.sync.dma_start(w2_sb[:], a["w2x"][:])
                nc.gpsimd.dma_start(x0_sb[:, 0:4], a["x0T"][:, 0:4])
                nc.gpsimd.dma_start(x0_sb[:, 4:8], a["x0T"][:, 4:8])
                nc.scalar.dma_start(ws_sb[:], a["wsx"][:])
                nc.scalar.dma_start(w3_sb[:], a["w3x"][:])
            elif DMAV == "q3g":
                nc.sync.dma_start(w1_sb[:, 0:2], a["w1x"][:, 0:2])
                nc.sync.dma_start(x0_sb[:, 8:13], a["x0T"][:, 8:13])
                nc.sync.dma_start(w1_sb[:, 2:4], a["w1x"][:, 2:4])
                nc.sync.dma_start(w1_sb[:, 4:6], a["w1x"][:, 4:6])
                nc.sync.dma_start(w1_sb[:, 6:8], a["w1x"][:, 6:8])
                nc.sync.dma_start(w2_sb[:], a["w2x"][:])
                nc.sync.dma_start(w3_sb[:], a["w3x"][:])
                nc.gpsimd.dma_start(x0_sb[:, 0:4], a["x0T"][:, 0:4])
                nc.gpsimd.dma_start(x0_sb[:, 4:8], a["x0T"][:, 4:8])
                nc.scalar.dma_start(ws_sb[:], a["wsx"][:])
            elif DMAV == "q3h":
                nc.sync.dma_start(w1_sb[:, 0:2], a["w1x"][:, 0:2])
                nc.sync.dma_start(x0_sb[:, 4:8], a["x0T"][:, 4:8])
                nc.sync.dma_start(x0_sb[:, 8:13], a["x0T"][:, 8:13])
                nc.sync.dma_start(w1_sb[:, 2:4], a["w1x"][:, 2:4])
                nc.sync.dma_start(w1_sb[:, 4:6], a["w1x"][:, 4:6])
                nc.sync.dma_start(w1_sb[:, 6:8], a["w1x"][:, 6:8])
                nc.sync.dma_start(w2_sb[:], a["w2x"][:])
                nc.sync.dma_start(w3_sb[:], a["w3x"][:])
                nc.gpsimd.dma_start(x0_sb[:, 0:4], a["x0T"][:, 0:4])
                nc.scalar.dma_start(ws_sb[:], a["wsx"][:])
            elif DMAV == "q3i":
                nc.sync.dma_start(x0_sb[:, 8:13], a["x0T"][:, 8:13])
                nc.sync.dma_start(w1_sb[:, 0:2], a["w1x"][:, 0:2])
                nc.sync.dma_start(w1_sb[:, 2:4], a["w1x"][:, 2:4])
                nc.sync.dma_start(w1_sb[:, 4:6], a["w1x"][:, 4:6])
                nc.sync.dma_start(w1_sb[:, 6:8], a["w1x"][:, 6:8])
                nc.sync.dma_start(w2_sb[:], a["w2x"][:])
                nc.sync.dma_start(w3_sb[:], a["w3x"][:])
                nc.gpsimd.dma_start(x0_sb[:, 0:8], a["x0T"][:, 0:8])
                nc.scalar.dma_start(ws_sb[:], a["wsx"][:])
            elif DMAV == "q3c":
                nc.sync.dma_start(w1_sb[:, 0:2], a["w1x"][:, 0:2])
                nc.sync.dma_start(w1_sb[:, 2:4], a["w1x"][:, 2:4])
                nc.sync.dma_start(w1_sb[:, 4:6], a["w1x"][:, 4:6])
                nc.sync.dma_start(w1_sb[:, 6:8], a["w1x"][:, 6:8])
                nc.sync.dma_start(w2_sb[:], a["w2x"][:])
                nc.sync.dma_start(w3_sb[:], a["w3x"][:])
                nc.gpsimd.dma_start(x0_sb[:, 0:4], a["x0T"][:, 0:4])
                nc.gpsimd.dma_start(x0_sb[:, 4:8], a["x0T"][:, 4:8])
                nc.gpsimd.dma_start(x0_sb[:, 8:13], a["x0T"][:, 8:13])
                nc.scalar.dma_start(ws_sb[:], a["wsx"][:])
            else:
                raise ValueError(DMAV)
            if True:
                pass
            if not zb:
                cb_sb = cpool.tile([128, KT, 3], FP8, tag="cb")
                nc.scalar.dma_start(cb_sb[:], a["cbx"][:])
                bx_sb = cpool.tile([128, 16], F32, tag="bx")
                nc.scalar.dma_start(bx_sb[:], a["bx"][:])

            # ---------------- DVE warm work + PE pacers -------------------
            pz = apool.tile([128, 2, 2], FP8, tag="pz")
            nc.vector.memset(pz[:], 1.0)
            # zero pad plane for x0 (k-tile 13)
            nc.vector.memset(x0_sb[:, 13, :], 0.0)
            ch = apool.tile([1, 2], FP8, tag="chain")
            nc.vector.memset(ch[:], 1.0)
            at_ps = psat.tile([128, 16], F32, tag="at", name="atps")
            pac_ps = at_ps[0:2, 0:2]

            def pacer(lhs, rhs):
                nc.tensor.matmul(out=pac_ps, lhsT=lhs, rhs=rhs,
                                 start=True, stop=True, skip_group_check=True)

            # pacer chain keeps the PE p-state ramp alive from ~0.4us on:
            # gaps between PE activity must stay < ~2.5us or the clock resets
            pacer(pz[:, 0, :], pz[:, 0, :])
            for seg in range(3):
                for i in range(6):
                    nc.vector.tensor_scalar(
                        out=ch[:], in0=ch[:], scalar1=1.0, scalar2=None,
                        op0=ALU.mult,
                    )
                pacer(ch[0:1, :], ch[0:1, :])
            wfirst = MORDER[0] if L1REORD else 0
            pacer(w1_sb[:, wfirst, 0, 0:2], w1_sb[:, wfirst, 0, 0:2])
            if os.environ.get("K_XPACE", "0") == "1":
                pacer(x0_sb[:, 8, 0:2], x0_sb[:, 8, 0:2])
                pacer(x0_sb[:, 0, 0:2], x0_sb[:, 0, 0:2])

            # ---------------- pre-armed output DMA ------------------------
            # kv_writeback descriptors are generated early (data-independent);
            # the Tile framework defers the RAW dep on `res` to trigger_dma.
            KVOUT = os.environ.get("K_KVOUT", "0") == "1"
            res = apool.tile([128, 1, 4, 1], F32, tag="res")
            if KVOUT:
                kvi = apool.tile([128, 4], mybir.dt.int32, tag="kvi")
                nc.vector.memset(kvi[:], 0)
                nc.gpsimd.kv_writeback(
                    a["out"][:], res[:], kvi[:],
                    prepare_only=True,
                )

            # ---------------- helpers -------------------------------------
            def tower_tile(ps, w_sb, m, nkt, rhs_sb):
                npair = nkt // 2
                for j in range(npair):
                    nc.tensor.matmul(
                        out=ps[:], lhsT=w_sb[:, m, 2 * j:2 * j + 2, :],
                        rhs=rhs_sb[:, 2 * j:2 * j + 2, :],
                        start=(j == 0), stop=(j == npair - 1),
                        perf_mode=DR,
                    )

            def conv_relu(src, dst_ap_fn, bcol, split):
                # dst = relu(src/64 [+ bias]) in fp8; split across ACT/DVE
                if not zb:
                    nc.scalar.activation(
                        out=dst_ap_fn(0, BC), in_=src[:, 0:BC],
                        func=AF.Relu, scale=INV, bias=bx_sb[:, bcol:bcol + 1],
                    )
                    return
                if split == "act":
                    cuts = [(nc.scalar, 0, BC)]
                elif split == "dve":
                    cuts = [(nc.vector, 0, BC)]
                else:
                    cuts = [(nc.scalar, 0, 256), (nc.vector, 256, BC)]
                for eng, lo, hi in cuts:
                    if eng is nc.scalar:
                        nc.scalar.activation(
                            out=dst_ap_fn(lo, hi), in_=src[:, lo:hi],
                            func=AF.Relu, scale=INV,
                        )
                    else:
                        eng.tensor_scalar(
                            out=dst_ap_fn(lo, hi), in0=src[:, lo:hi],
                            scalar1=INV, scalar2=0.0,
                            op0=ALU.mult, op1=ALU.max,
                        )

            # ---------------- ACT warmup (table preload) ------------------
            warm1 = apool.tile([1, 1], F32, tag="warm1")
            nc.scalar.activation(out=warm1[:], in_=pz[0:1, 0, 0:1], func=AF.Relu)
            warm2 = apool.tile([1, 1], F32, tag="warm2")
            nc.scalar.activation(out=warm2[:], in_=pz[0:1, 0, 0:1],
                                 func=AF.Sigmoid)

            # ---------------- L1 first m-tile -----------------------------
            l1order = MORDER if L1REORD else list(range(M1))
            h1T = apool.tile([128, M1, BC], FP8, tag="h1T")
            m0 = l1order[0]
            ps = psmm.tile([128, BC], F32, tag="mm", name=f"l1_{m0}")
            tower_tile(ps, w1_sb, m0, KTP, x0_sb)
            conv_relu(ps, (lambda mm: lambda lo, hi: h1T[:, mm, lo:hi])(m0),
                      m0, "half")

            # ---------------- at: a_j with examples on partitions ---------
            first = True
            for c in range(4):
                sl = slice(4 * c, 4 * c + 4)
                ech = slice(128 * c, 128 * c + 128)
                for j in range(KP1):
                    nc.tensor.matmul(
                        out=at_ps[:, sl],
                        lhsT=x0_sb[:, 2 * j:2 * j + 2, ech],
                        rhs=ws_sb[:, 2 * j:2 * j + 2, :],
                        start=first, stop=(c == 3 and j == KP1 - 1),
                        perf_mode=DR, skip_group_check=True,
                    )
                    first = False

            # cb scalars (general-bias path only)
            if not zb:
                cb_ps = pslg.tile([4, 3], F32, tag="lg", name="cbps")
                for j in range(KT // 2):
                    nc.tensor.matmul(
                        out=cb_ps[:], lhsT=ws_sb[:, 2 * j:2 * j + 2, :],
                        rhs=cb_sb[:, 2 * j:2 * j + 2, :],
                        start=(j == 0), stop=False, perf_mode=DR,
                    )
                nc.tensor.matmul(
                    out=cb_ps[:], lhsT=ws_sb[:, KT - 1, :],
                    rhs=cb_sb[:, KT - 1, :], start=False, stop=True,
                )
                cbs = apool.tile([4, 3], F32, tag="cbs")
                nc.vector.tensor_scalar(
                    out=cbs[:], in0=cb_ps[:], scalar1=INV2, scalar2=None,
                    op0=ALU.mult,
                )
                cbf = apool.tile([1, 12], F32, tag="cbf")
                nc.sync.dma_start(cbf[:], cbs[:])
                stage = apool.tile([1, 3], F32, tag="stage")
                nc.vector.tensor_copy(out=stage[:, 0:1], in_=cbf[:, 3:4])
                nc.vector.reduce_sum(out=stage[:, 1:2], in_=cbf[:, 6:8], axis=AX.X)
                r3 = apool.tile([1, 1], F32, tag="r3")
                nc.vector.reduce_sum(out=r3[:], in_=cbf[:, 9:12], axis=AX.X)
                nc.vector.tensor_scalar(
                    out=stage[:, 2:3], in0=r3[:], scalar1=bx_sb[0:1, 14:15],
                    scalar2=None, op0=ALU.add,
                )
                csc = apool.tile([128, 3], F32, tag="csc")
                nc.sync.dma_start(csc[:], stage[0:1, :].partition_broadcast(128))

            # ---------------- L1 remaining m-tiles ------------------------
            for mi in range(1, M1):
                m = l1order[mi]
                ps = psmm.tile([128, BC], F32, tag="mm", name=f"l1_{m}")
                tower_tile(ps, w1_sb, m, KTP, x0_sb)
                if mi == 1:
                    # att copies + early recurrence on DVE, between convs
                    att = apool.tile([128, 4, 4], F32, tag="att")
                    for c in range(4):
                        nc.vector.tensor_scalar(
                            out=att[:, :, c], in0=at_ps[:, 4 * c:4 * c + 4],
                            scalar1=INV2, scalar2=None, op0=ALU.mult,
                        )
                    u1 = apool.tile([128, 4], F32, tag="u1")
                    nc.vector.tensor_scalar(
                        out=u1[:], in0=att[:, 1, :], scalar1=1.0,
                        scalar2=None, op0=ALU.add,
                    )
                    k1 = apool.tile([128, 4], F32, tag="k1")
                    nc.vector.scalar_tensor_tensor(
                        out=k1[:], in0=att[:, 2, :], scalar=1.0,
                        in1=att[:, 3, :], op0=ALU.add, op1=ALU.mult,
                    )
                    z = apool.tile([128, 4], F32, tag="z")
                    nc.vector.scalar_tensor_tensor(
                        out=z[:], in0=att[:, 0, :], scalar=1.0,
                        in1=u1[:], op0=ALU.add, op1=ALU.mult,
                    )
                    mz = apool.tile([128, 4], F32, tag="mz")
                    if zb:
                        nc.vector.tensor_mul(mz[:], z[:], k1[:])
                    else:
                        nc.vector.scalar_tensor_tensor(
                            out=mz[:], in0=z[:], scalar=csc[:, 0:1],
                            in1=k1[:], op0=ALU.add, op1=ALU.mult,
                        )
                        mz2 = apool.tile([128, 4], F32, tag="mz2")
                        nc.vector.scalar_tensor_tensor(
                            out=mz2[:], in0=att[:, 3, :], scalar=csc[:, 1:2],
                            in1=mz[:], op0=ALU.mult, op1=ALU.add,
                        )
                        mz = mz2
                conv_relu(ps, (lambda mm: lambda lo, hi: h1T[:, mm, lo:hi])(m),
                          m, "half")

            # ---------------- L2 (2-bank pool forces m-serial) ------------
            h2T = apool.tile([128, M2, BC], FP8, tag="h2T")
            for m in range(M2):
                ps = psmm.tile([128, BC], F32, tag="mm", name=f"l2_{m}")
                tower_tile(ps, w2_sb, m, K2T, h1T)
                conv_relu(ps, (lambda mm: lambda lo, hi: h2T[:, mm, lo:hi])(m),
                          M1 + m,
                          L2CONV if L2CONV != "alt"
                          else ("act" if m % 2 == 0 else "dve"))

            # ---------------- L3 (chunk-aligned conv quarters) ------------
            h3T = apool.tile([128, M3, BC], FP8, tag="h3T")
            for m in range(M3):
                ps = psmm.tile([128, BC], F32, tag="mm", name=f"l3_{m}")
                tower_tile(ps, w3_sb, m, K3T, h2T)
                h3cv = os.environ.get("K_H3CONV", "whole")
                if zb and h3cv == "quart":
                    for ci in range(4):
                        lo, hi = 128 * ci, 128 * ci + 128
                        eng = nc.scalar if ci < 2 else nc.vector
                        if ci < 2:
                            nc.scalar.activation(
                                out=h3T[:, m, lo:hi], in_=ps[:, lo:hi],
                                func=AF.Relu, scale=INV,
                            )
                        else:
                            nc.vector.tensor_scalar(
                                out=h3T[:, m, lo:hi], in0=ps[:, lo:hi],
                                scalar1=INV, scalar2=0.0,
                                op0=ALU.mult, op1=ALU.max,
                            )
                elif zb and h3cv == "whole":
                    conv_relu(
                        ps,
                        (lambda mm: lambda lo, hi: h3T[:, mm, lo:hi])(m),
                        M1 + M2 + m, "act" if m == 0 else "dve")
                else:
                    conv_relu(
                        ps,
                        (lambda mm: lambda lo, hi: h3T[:, mm, lo:hi])(m),
                        M1 + M2 + m, "half")

            # ---------------- head: lg with examples on partitions --------
            # mz (ready early) is pre-written into the head psum scaled by
            # 4096; head matmuls accumulate on top (start=False throughout),
            # so sigmoid reads the finished logit*4096 straight from PSUM.
            MZPS = os.environ.get("K_MZPS", "1") == "1"
            lg_ps = pslg.tile([128, 4], F32, tag="lg", name="lgps")
            if MZPS:
                nc.vector.tensor_scalar(
                    out=lg_ps[:], in0=mz[:], scalar1=float(S * S),
                    scalar2=None, op0=ALU.mult,
                )
            for kt in range(KHT):
                for c in range(4):
                    nc.tensor.matmul(
                        out=lg_ps[:, c:c + 1],
                        lhsT=h3T[:, kt, 128 * c:128 * c + 128],
                        rhs=ws_sb[:, 14, kt:kt + 1],
                        start=(not MZPS and kt == 0 and c == 0),
                        stop=(kt == KHT - 1 and c == 3),
                        skip_group_check=True,
                    )

            # ---------------- sigmoid + store -----------------------------
            res_ap = res[:, 0, :, 0]
            if zb:
                nc.scalar.activation(out=res_ap, in_=lg_ps[:],
                                     func=AF.Sigmoid, scale=INV2)
            else:
                nc.scalar.activation(
                    out=res_ap, in_=lg_ps[:], func=AF.Sigmoid, scale=INV2,
                    bias=csc[:, 2:3],
                )
            if KVOUT:
                nc.gpsimd.trigger_dma(count=None)
            else:
                nc.sync.dma_start(a["outf"][:], res_ap)

        if R == 1:
            _body()
        else:
            with tc.For_i(0, R, 1):
                _body()


def build_program(zb):
    key = ("nc", zb, os.environ.get("K_REPEAT", "1"))
    if key in _CACHE:
        return _CACHE[key]
    nc = bacc.Bacc("TRN2", target_bir_lowering=False, debug=False, num_devices=NC)
    aps = {}

    def din(name, shape, dt):
        aps[name] = nc.dram_tensor(name, shape, dt, kind="ExternalInput").ap()

    din("x0T", [128, KT, BC], FP8)
    din("w1x", [128, M1, KTP, 128], FP8)
    din("w2x", [128, M2, K2T, 128], FP8)
    din("w3x", [128, M3, K3T, 128], FP8)
    din("wsx", [128, 15, 4], FP8)
    if not zb:
        din("cbx", [128, KT, 3], FP8)
        din("bx", [128, 16], F32)
    if os.environ.get("K_KVOUT", "0") == "1":
        aps["out"] = nc.dram_tensor(
            "out", [4, 128, 1, 1], F32, kind="ExternalOutput").ap()
    else:
        aps["outf"] = nc.dram_tensor(
            "outf", [128, 4], F32, kind="ExternalOutput").ap()
    _CACHE["aps"] = aps

    with tile.TileContext(nc) as tc:
        _emit(tc, zb)
    nc.compile()
    _CACHE[key] = nc
    return nc


def prepare_in_maps(inputs):
    import ml_dtypes
    f8 = ml_dtypes.float8_e4m3fn

    ids = np.asarray(inputs["ids"]).astype(np.int64)
    emb = np.asarray(inputs["emb"], dtype=np.float32)
    cross_w = np.asarray(inputs["cross_w"], dtype=np.float32)
    cross_b = np.asarray(inputs["cross_b"], dtype=np.float32)
    w1 = np.asarray(inputs["w1"], dtype=np.float32)
    w2 = np.asarray(inputs["w2"], dtype=np.float32)
    w3 = np.asarray(inputs["w3"], dtype=np.float32)
    b1 = np.asarray(inputs["b1"], dtype=np.float32)
    b2 = np.asarray(inputs["b2"], dtype=np.float32)
    b3 = np.asarray(inputs["b3"], dtype=np.float32)
    out_w = np.asarray(inputs["out_w"], dtype=np.float32)
    out_b = np.float32(np.asarray(inputs["out_b"], dtype=np.float32))

    zb = not (
        np.any(cross_b) or np.any(b1) or np.any(b2) or np.any(b3)
        or float(out_b) != 0.0
    )

    def lhst(w, ktiles, mtiles, kpad=0):
        # [K, M] -> [128, mtiles, ktiles+kpad, 128], scaled fp8
        arr = (w * S).astype(f8).reshape(ktiles, 128, mtiles, 128) \
            .transpose(1, 2, 0, 3)
        if kpad:
            z = np.zeros((128, mtiles, kpad, 128), f8)
            arr = np.concatenate([arr, z], axis=2)
        return np.ascontiguousarray(arr)

    w4 = np.concatenate([cross_w.T, out_w[:D].reshape(D, 1)], axis=1)  # [D,4]
    wsx = np.zeros((128, 15, 4), f8)
    wsx[:, 0:KT, :] = (w4 * S).astype(f8).reshape(KT, 128, 4).transpose(1, 0, 2)
    wsx[:, 14, 0:KHT] = (out_w[D:, 0] * S).astype(f8).reshape(KHT, 128).T
    shared = dict(
        w1x=lhst(w1, KT, M1, kpad=1),
        w2x=lhst(w2, K2T, M2),
        w3x=lhst(w3, K3T, M3),
        wsx=np.ascontiguousarray(wsx),
    )
    if not zb:
        shared["cbx"] = np.ascontiguousarray(
            (cross_b.T * S).astype(f8).reshape(KT, 128, 3).transpose(1, 0, 2)
        )
        bx = np.zeros((128, 16), np.float32)
        bx[:, 0:M1] = (b1 * S).reshape(M1, 128).T
        bx[:, M1:M1 + M2] = (b2 * S).reshape(M2, 128).T
        bx[:, M1 + M2:M1 + M2 + M3] = (b3 * S).reshape(M3, 128).T
        bx[0, 14] = out_b
        shared["bx"] = bx

    emb8 = (emb * S).astype(f8)
    in_maps = []
    for c in range(NC):
        idsc = ids[c * BC:(c + 1) * BC].reshape(-1)
        x0c = emb8[idsc].reshape(BC, D)                   # [512, 1664] fp8
        x0T = np.ascontiguousarray(
            x0c.reshape(BC, KT, 128).transpose(2, 1, 0)   # [128, 13, 512]
        )
        in_maps.append(dict(x0T=x0T, **shared))
    return in_maps, zb


def kernel(**inputs):
    in_maps, zb = prepare_in_maps(inputs)
    nc = build_program(zb)
    res = bass_utils.run_bass_kernel_spmd(nc, in_maps, core_ids=list(range(NC)))
    key = "out" if os.environ.get("K_KVOUT", "0") == "1" else "outf"
    out = np.concatenate(
        [res.results[c][key].reshape(4, 128) if key == "out"
         else res.results[c][key].T.reshape(BC)
         for c in range(NC)], axis=None,
    ).reshape(B, 1)
    return out.astype(np.float32)


# revision 5
# speedup vs baseline: 1.0016x; 1.0016x over previous
"""Deep & Cross Network kernel for 8x Trainium2 NeuronCores (Bass/Tile).

Sharding: pure data-parallel over batch (512 rows/core); weights replicated
per core; no collectives.

Host-side prep inside kernel() (data movement / layout / dtype casts only):
  - embedding lookup x0 = emb[ids] (pure gather), scale by 64, cast fp8e4m3
  - pre-tile weights (scaled by 64, fp8) into SBUF-native DoubleRow layouts

Device math (per core, batch n=512, D=1664), all matmuls fp8 DoubleRow
(0.5 cycles/row, 256-deep contraction per instruction):
  - Deep tower fully transposed: H_l = relu(PSUM/64) with PSUM = (64x)(64w)
    so every intermediate stays in fp8-friendly range. D padded to 14 k-tiles
    (zero plane) so L1 is pure DoubleRow.
  - a_j = x0 . w4_j (w4 = [cross_w_i, out_w[:D]]) computed with examples on
    PARTITIONS by using the x0T slice as the stationary operand -> at [128,16]
  - head lg = ow2 . h3 likewise -> lg_ps [128, 4]
  - cross net collapses algebraically: with u_j = 1 + a_j,
      t3 = (u0 u1 + c1) u2 + c2,  logit = t3*a3 + C + lg + out_b
    (c1, c2, C are cross_b-derived scalars; zero when cross_b == 0)
  - recurrence runs on [128, 4] tiles (examples on partitions).
Early tiny "pacer" matmuls keep the PE p-state ramp alive so real matmuls
run at full clock from the start.
"""

import sys
import os
import numpy as np

for _p in ("/opt/trn_rl_repo",):
    if _p not in sys.path:
        sys.path.insert(0, _p)

import concourse.bass as bass
import concourse.tile as tile
from concourse import bacc, mybir
from concourse import bass_utils

F32 = mybir.dt.float32
FP8 = mybir.dt.float8e4
AF = mybir.ActivationFunctionType
ALU = mybir.AluOpType
AX = mybir.AxisListType
DR = mybir.MatmulPerfMode.DoubleRow

B, F, E, H = 4096, 26, 64, 1000000
D = F * E            # 1664
NC = 8
BC = B // NC         # 512 rows per core
KT = D // 128        # 13 k-tiles over D
KTP = KT + 1         # padded to 14 (zero plane) -> 7 DoubleRow pairs
KP1 = KTP // 2
H1, H2, H3 = 1024, 512, 256
M1, M2, M3 = H1 // 128, H2 // 128, H3 // 128
K2T, K3T, KHT = H1 // 128, H2 // 128, H3 // 128
S = 64.0             # fp8 range scale
INV = float(1.0 / S)
INV2 = float(1.0 / (S * S))

_CACHE = {}


def _emit(tc, zb):
    nc = tc.nc
    a = _CACHE["aps"]
    R = int(os.environ.get("K_REPEAT", "1"))
    L2CONV = os.environ.get("K_L2CONV", "rev")
    DMAV = os.environ.get("K_DMAV", "q2kv")
    L1REORD = os.environ.get("K_L1REORD", "0") == "1"
    MORDER = [6, 7, 0, 1, 2, 3, 4, 5]

    with (
        tc.tile_pool(name="const", bufs=1) as cpool,
        tc.tile_pool(name="act", bufs=1) as apool,
        tc.tile_pool(name="psmm", bufs=6, space="PSUM") as psmm,
        tc.tile_pool(name="psat", bufs=1, space="PSUM") as psat,
        tc.tile_pool(name="pslg", bufs=1, space="PSUM") as pslg,
    ):
        def _body():
            # ---------------- DMA streams (order = priority) --------------
            # Three DMA queues in parallel (each has ~1.28us start cadence):
            # SP: w1/w2 in ~1274ns paired chunks; gpsimd (SWDGE): x0 stream;
            # ACT: tiny/slack transfers.
            w1_sb = cpool.tile([128, M1, KTP, 128], FP8, tag="w1")
            x0_sb = cpool.tile([128, KTP, BC], FP8, tag="x0")
            w2_sb = cpool.tile([128, M2, K2T, 128], FP8, tag="w2")
            w3_sb = cpool.tile([128, M3, K3T, 128], FP8, tag="w3")
            ws_sb = cpool.tile([128, 15, 4], FP8, tag="ws")
            if DMAV == "q3":
                nc.sync.dma_start(w1_sb[:, 0:2], a["w1x"][:, 0:2])
                nc.sync.dma_start(w1_sb[:, 2:4], a["w1x"][:, 2:4])
                nc.sync.dma_start(w1_sb[:, 4:6], a["w1x"][:, 4:6])
                nc.sync.dma_start(w1_sb[:, 6:8], a["w1x"][:, 6:8])
                nc.sync.dma_start(w2_sb[:], a["w2x"][:])
                nc.sync.dma_start(w3_sb[:], a["w3x"][:])
                nc.gpsimd.dma_start(x0_sb[:, 0:8], a["x0T"][:, 0:8])
                nc.gpsimd.dma_start(x0_sb[:, 8:13], a["x0T"][:, 8:13])
                nc.scalar.dma_start(ws_sb[:], a["wsx"][:])
            elif DMAV == "q3b":
                worder = MORDER if L1REORD else [0, 1, 2, 3, 4, 5, 6, 7]
                for i in range(0, 8, 2):
                    lo = worder[i]
                    nc.sync.dma_start(w1_sb[:, lo:lo + 2], a["w1x"][:, lo:lo + 2])
                    if i == 0:
                        nc.sync.dma_start(x0_sb[:, 8:13], a["x0T"][:, 8:13])
                nc.sync.dma_start(w2_sb[:], a["w2x"][:])
                nc.sync.dma_start(w3_sb[:], a["w3x"][:])
                nc.gpsimd.dma_start(x0_sb[:, 0:8], a["x0T"][:, 0:8])
                nc.scalar.dma_start(ws_sb[:], a["wsx"][:])
            elif DMAV == "v3":
                nc.scalar.dma_start(ws_sb[:], a["wsx"][:])
                nc.scalar.dma_start(x0_sb[:, 12:13], a["x0T"][:, 12:13])
                nc.sync.dma_start(w1_sb[:, 0], a["w1x"][:, 0])
                nc.sync.dma_start(x0_sb[:, 0:4], a["x0T"][:, 0:4])
                nc.sync.dma_start(x0_sb[:, 4:8], a["x0T"][:, 4:8])
                nc.sync.dma_start(x0_sb[:, 8:12], a["x0T"][:, 8:12])
                nc.sync.dma_start(w1_sb[:, 1:3], a["w1x"][:, 1:3])
                nc.sync.dma_start(w1_sb[:, 3:5], a["w1x"][:, 3:5])
                nc.sync.dma_start(w1_sb[:, 5:7], a["w1x"][:, 5:7])
                nc.sync.dma_start(w1_sb[:, 7:8], a["w1x"][:, 7:8])
                nc.sync.dma_start(w2_sb[:, 0:2], a["w2x"][:, 0:2])
                nc.sync.dma_start(w2_sb[:, 2:4], a["w2x"][:, 2:4])
                nc.sync.dma_start(w3_sb[:], a["w3x"][:])
            elif DMAV == "q2kv":
                nc.sync.dma_start(w1_sb[:, 0:2], a["w1x"][:, 0:2])
                nc.sync.dma_start(x0_sb[:, 8:13], a["x0T"][:, 8:13])
                nc.sync.dma_start(w1_sb[:, 2:4], a["w1x"][:, 2:4])
                nc.sync.dma_start(w1_sb[:, 4:6], a["w1x"][:, 4:6])
                nc.sync.dma_start(w1_sb[:, 6:8], a["w1x"][:, 6:8])
                nc.sync.dma_start(w2_sb[:], a["w2x"][:])
                nc.sync.dma_start(w3_sb[:], a["w3x"][:])
                nc.scalar.dma_start(x0_sb[:, 0:8], a["x0T"][:, 0:8])
                nc.scalar.dma_start(ws_sb[:], a["wsx"][:])
            elif DMAV == "q3j":
                nc.sync.dma_start(w1_sb[:, 0:2], a["w1x"][:, 0:2])
                nc.sync.dma_start(x0_sb[:, 8:13], a["x0T"][:, 8:13])
                nc.sync.dma_start(w1_sb[:, 2:4], a["w1x"][:, 2:4])
                nc.sync.dma_start(w1_sb[:, 4:6], a["w1x"][:, 4:6])
                nc.sync.dma_start(w1_sb[:, 6:8], a["w1x"][:, 6:8])
                nc.sync.dma_start(w2_sb[:], a["w2x"][:])
                nc.sync.dma_start(w3_sb[:], a["w3x"][:])
                nc.gpsimd.dma_start(x0_sb[:, 2:8], a["x0T"][:, 2:8])
                nc.scalar.dma_start(x0_sb[:, 0:2], a["x0T"][:, 0:2])
                nc.scalar.dma_start(ws_sb[:], a["wsx"][:])
            elif DMAV == "q3k":
                nc.sync.dma_start(w1_sb[:, 0:2], a["w1x"][:, 0:2])
                nc.sync.dma_start(x0_sb[:, 8:13], a["x0T"][:, 8:13])
                nc.sync.dma_start(w1_sb[:, 2:4], a["w1x"][:, 2:4])
                nc.sync.dma_start(w1_sb[:, 4:6], a["w1x"][:, 4:6])
                nc.sync.dma_start(w1_sb[:, 6:8], a["w1x"][:, 6:8])
                nc.sync.dma_start(w2_sb[:], a["w2x"][:])
                nc.gpsimd.dma_start(x0_sb[:, 0:4], a["x0T"][:, 0:4])
                nc.gpsimd.dma_start(x0_sb[:, 4:8], a["x0T"][:, 4:8])
                nc.scalar.dma_start(ws_sb[:], a["wsx"][:])
                nc.scalar.dma_start(w3_sb[:], a["w3x"][:])
            elif DMAV == "q3g":
                nc.sync.dma_start(w1_sb[:, 0:2], a["w1x"][:, 0:2])
                nc.sync.dma_start(x0_sb[:, 8:13], a["x0T"][:, 8:13])
                nc.sync.dma_start(w1_sb[:, 2:4], a["w1x"][:, 2:4])
                nc.sync.dma_start(w1_sb[:, 4:6], a["w1x"][:, 4:6])
                nc.sync.dma_start(w1_sb[:, 6:8], a["w1x"][:, 6:8])
                nc.sync.dma_start(w2_sb[:], a["w2x"][:])
                nc.sync.dma_start(w3_sb[:], a["w3x"][:])
                nc.gpsimd.dma_start(x0_sb[:, 0:4], a["x0T"][:, 0:4])
                nc.gpsimd.dma_start(x0_sb[:, 4:8], a["x0T"][:, 4:8])
                nc.scalar.dma_start(ws_sb[:], a["wsx"][:])
            elif DMAV == "q3h":
                nc.sync.dma_start(w1_sb[:, 0:2], a["w1x"][:, 0:2])
                nc.sync.dma_start(x0_sb[:, 4:8], a["x0T"][:, 4:8])
                nc.sync.dma_start(x0_sb[:, 8:13], a["x0T"][:, 8:13])
                nc.sync.dma_start(w1_sb[:, 2:4], a["w1x"][:, 2:4])
                nc.sync.dma_start(w1_sb[:, 4:6], a["w1x"][:, 4:6])
                nc.sync.dma_start(w1_sb[:, 6:8], a["w1x"][:, 6:8])
                nc.sync.dma_start(w2_sb[:], a["w2x"][:])
                nc.sync.dma_start(w3_sb[:], a["w3x"][:])
                nc.gpsimd.dma_start(x0_sb[:, 0:4], a["x0T"][:, 0:4])
                nc.scalar.dma_start(ws_sb[:], a["wsx"][:])
            elif DMAV == "q3i":
                nc.sync.dma_start(x0_sb[:, 8:13], a["x0T"][:, 8:13])
                nc.sync.dma_start(w1_sb[:, 0:2], a["w1x"][:, 0:2])
                nc.sync.dma_start(w1_sb[:, 2:4], a["w1x"][:, 2:4])
                nc.sync.dma_start(w1_sb[:, 4:6], a["w1x"][:, 4:6])
                nc.sync.dma_start(w1_sb[:, 6:8], a["w1x"][:, 6:8])
                nc.sync.dma_start(w2_sb[:], a["w2x"][:])
                nc.sync.dma_start(w3_sb[:], a["w3x"][:])
                nc.gpsimd.dma_start(x0_sb[:, 0:8], a["x0T"][:, 0:8])
                nc.scalar.dma_start(ws_sb[:], a["wsx"][:])
            elif DMAV == "q3c":
                nc.sync.dma_start(w1_sb[:, 0:2], a["w1x"][:, 0:2])
                nc.sync.dma_start(w1_sb[:, 2:4], a["w1x"][:, 2:4])
                nc.sync.dma_start(w1_sb[:, 4:6], a["w1x"][:, 4:6])
                nc.sync.dma_start(w1_sb[:, 6:8], a["w1x"][:, 6:8])
                nc.sync.dma_start(w2_sb[:], a["w2x"][:])
                nc.sync.dma_start(w3_sb[:], a["w3x"][:])
                nc.gpsimd.dma_start(x0_sb[:, 0:4], a["x0T"][:, 0:4])
                nc.gpsimd.dma_start(x0_sb[:, 4:8], a["x0T"][:, 4:8])
                nc.gpsimd.dma_start(x0_sb[:, 8:13], a["x0T"][:, 8:13])
                nc.scalar.dma_start(ws_sb[:], a["wsx"][:])
            else:
                raise ValueError(DMAV)
            if True:
                pass
            if not zb:
                cb_sb = cpool.tile([128, KT, 3], FP8, tag="cb")
                nc.scalar.dma_start(cb_sb[:], a["cbx"][:])
                bx_sb = cpool.tile([128, 16], F32, tag="bx")
                nc.scalar.dma_start(bx_sb[:], a["bx"][:])

            # ---------------- DVE warm work + PE pacers -------------------
            pz = apool.tile([128, 2, 2], FP8, tag="pz")
            nc.vector.memset(pz[:], 1.0)
            # zero pad plane for x0 (k-tile 13)
            nc.vector.memset(x0_sb[:, 13, :], 0.0)
            ch = apool.tile([1, 2], FP8, tag="chain")
            nc.vector.memset(ch[:], 1.0)
            at_ps = psat.tile([128, 16], F32, tag="at", name="atps")
            pac_ps = at_ps[0:2, 0:2]

            def pacer(lhs, rhs):
                nc.tensor.matmul(out=pac_ps, lhsT=lhs, rhs=rhs,
                                 start=True, stop=True, skip_group_check=True)

            # pacer chain keeps the PE p-state ramp alive from ~0.4us on:
            # gaps between PE activity must stay < ~2.5us or the clock resets
            pacer(pz[:, 0, :], pz[:, 0, :])
            for seg in range(3):
                for i in range(6):
                    nc.vector.tensor_scalar(
                        out=ch[:], in0=ch[:], scalar1=1.0, scalar2=None,
                        op0=ALU.mult,
                    )
                pacer(ch[0:1, :], ch[0:1, :])
            wfirst = MORDER[0] if L1REORD else 0
            pacer(w1_sb[:, wfirst, 0, 0:2], w1_sb[:, wfirst, 0, 0:2])
            if os.environ.get("K_XPACE", "0") == "1":
                pacer(x0_sb[:, 8, 0:2], x0_sb[:, 8, 0:2])
                pacer(x0_sb[:, 0, 0:2], x0_sb[:, 0, 0:2])

            # ---------------- pre-armed output DMA ------------------------
            # kv_writeback descriptors are generated early (data-independent);
            # the Tile framework defers the RAW dep on `res` to trigger_dma.
            KVOUT = os.environ.get("K_KVOUT", "0") == "1"
            res = apool.tile([128, 1, 4, 1], F32, tag="res")
            if KVOUT:
                kvi = apool.tile([128, 4], mybir.dt.int32, tag="kvi")
                nc.vector.memset(kvi[:], 0)
                nc.gpsimd.kv_writeback(
                    a["out"][:], res[:], kvi[:],
                    prepare_only=True,
                )

            # ---------------- helpers -------------------------------------
            def tower_tile(ps, w_sb, m, nkt, rhs_sb):
                npair = nkt // 2
                for j in range(npair):
                    nc.tensor.matmul(
                        out=ps[:], lhsT=w_sb[:, m, 2 * j:2 * j + 2, :],
                        rhs=rhs_sb[:, 2 * j:2 * j + 2, :],
                        start=(j == 0), stop=(j == npair - 1),
                        perf_mode=DR,
                    )

            def conv_relu(src, dst_ap_fn, bcol, split):
                # dst = relu(src/64 [+ bias]) in fp8; split across ACT/DVE
                if not zb:
                    nc.scalar.activation(
                        out=dst_ap_fn(0, BC), in_=src[:, 0:BC],
                        func=AF.Relu, scale=INV, bias=bx_sb[:, bcol:bcol + 1],
                    )
                    return
                if split == "act":
                    cuts = [(nc.scalar, 0, BC)]
                elif split == "dve":
                    cuts = [(nc.vector, 0, BC)]
                else:
                    cuts = [(nc.scalar, 0, 256), (nc.vector, 256, BC)]
                for eng, lo, hi in cuts:
                    if eng is nc.scalar:
                        nc.scalar.activation(
                            out=dst_ap_fn(lo, hi), in_=src[:, lo:hi],
                            func=AF.Relu, scale=INV,
                        )
                    else:
                        eng.tensor_scalar(
                            out=dst_ap_fn(lo, hi), in0=src[:, lo:hi],
                            scalar1=INV, scalar2=0.0,
                            op0=ALU.mult, op1=ALU.max,
                        )

            # ---------------- ACT warmup (table preload) ------------------
            warm1 = apool.tile([1, 1], F32, tag="warm1")
            nc.scalar.activation(out=warm1[:], in_=pz[0:1, 0, 0:1], func=AF.Relu)
            warm2 = apool.tile([1, 1], F32, tag="warm2")
            nc.scalar.activation(out=warm2[:], in_=pz[0:1, 0, 0:1],
                                 func=AF.Sigmoid)

            # ---------------- L1 first m-tile -----------------------------
            l1order = MORDER if L1REORD else list(range(M1))
            h1T = apool.tile([128, M1, BC], FP8, tag="h1T")
            m0 = l1order[0]
            ps = psmm.tile([128, BC], F32, tag="mm", name=f"l1_{m0}")
            tower_tile(ps, w1_sb, m0, KTP, x0_sb)
            conv_relu(ps, (lambda mm: lambda lo, hi: h1T[:, mm, lo:hi])(m0),
                      m0, "half")

            # ---------------- at: a_j with examples on partitions ---------
            first = True
            for c in range(4):
                sl = slice(4 * c, 4 * c + 4)
                ech = slice(128 * c, 128 * c + 128)
                for j in range(KP1):
                    nc.tensor.matmul(
                        out=at_ps[:, sl],
                        lhsT=x0_sb[:, 2 * j:2 * j + 2, ech],
                        rhs=ws_sb[:, 2 * j:2 * j + 2, :],
                        start=first, stop=(c == 3 and j == KP1 - 1),
                        perf_mode=DR, skip_group_check=True,
                    )
                    first = False

            # cb scalars (general-bias path only)
            if not zb:
                cb_ps = pslg.tile([4, 3], F32, tag="lg", name="cbps")
                for j in range(KT // 2):
                    nc.tensor.matmul(
                        out=cb_ps[:], lhsT=ws_sb[:, 2 * j:2 * j + 2, :],
                        rhs=cb_sb[:, 2 * j:2 * j + 2, :],
                        start=(j == 0), stop=False, perf_mode=DR,
                    )
                nc.tensor.matmul(
                    out=cb_ps[:], lhsT=ws_sb[:, KT - 1, :],
                    rhs=cb_sb[:, KT - 1, :], start=False, stop=True,
                )
                cbs = apool.tile([4, 3], F32, tag="cbs")
                nc.vector.tensor_scalar(
                    out=cbs[:], in0=cb_ps[:], scalar1=INV2, scalar2=None,
                    op0=ALU.mult,
                )
                cbf = apool.tile([1, 12], F32, tag="cbf")
                nc.sync.dma_start(cbf[:], cbs[:])
                stage = apool.tile([1, 3], F32, tag="stage")
                nc.vector.tensor_copy(out=stage[:, 0:1], in_=cbf[:, 3:4])
                nc.vector.reduce_sum(out=stage[:, 1:2], in_=cbf[:, 6:8], axis=AX.X)
                r3 = apool.tile([1, 1], F32, tag="r3")
                nc.vector.reduce_sum(out=r3[:], in_=cbf[:, 9:12], axis=AX.X)
                nc.vector.tensor_scalar(
                    out=stage[:, 2:3], in0=r3[:], scalar1=bx_sb[0:1, 14:15],
                    scalar2=None, op0=ALU.add,
                )
                csc = apool.tile([128, 3], F32, tag="csc")
                nc.sync.dma_start(csc[:], stage[0:1, :].partition_broadcast(128))

            # ---------------- L1 remaining m-tiles ------------------------
            for mi in range(1, M1):
                m = l1order[mi]
                ps = psmm.tile([128, BC], F32, tag="mm", name=f"l1_{m}")
                tower_tile(ps, w1_sb, m, KTP, x0_sb)
                if mi == 1:
                    # att copies + early recurrence on DVE, between convs
                    att = apool.tile([128, 4, 4], F32, tag="att")
                    for c in range(4):
                        nc.vector.tensor_scalar(
                            out=att[:, :, c], in0=at_ps[:, 4 * c:4 * c + 4],
                            scalar1=INV2, scalar2=None, op0=ALU.mult,
                        )
                    u1 = apool.tile([128, 4], F32, tag="u1")
                    nc.vector.tensor_scalar(
                        out=u1[:], in0=att[:, 1, :], scalar1=1.0,
                        scalar2=None, op0=ALU.add,
                    )
                    k1 = apool.tile([128, 4], F32, tag="k1")
                    nc.vector.scalar_tensor_tensor(
                        out=k1[:], in0=att[:, 2, :], scalar=1.0,
                        in1=att[:, 3, :], op0=ALU.add, op1=ALU.mult,
                    )
                    z = apool.tile([128, 4], F32, tag="z")
                    nc.vector.scalar_tensor_tensor(
                        out=z[:], in0=att[:, 0, :], scalar=1.0,
                        in1=u1[:], op0=ALU.add, op1=ALU.mult,
                    )
                    mz = apool.tile([128, 4], F32, tag="mz")
                    if zb:
                        nc.vector.tensor_mul(mz[:], z[:], k1[:])
                    else:
                        nc.vector.scalar_tensor_tensor(
                            out=mz[:], in0=z[:], scalar=csc[:, 0:1],
                            in1=k1[:], op0=ALU.add, op1=ALU.mult,
                        )
                        mz2 = apool.tile([128, 4], F32, tag="mz2")
                        nc.vector.scalar_tensor_tensor(
                            out=mz2[:], in0=att[:, 3, :], scalar=csc[:, 1:2],
                            in1=mz[:], op0=ALU.mult, op1=ALU.add,
                        )
                        mz = mz2
                conv_relu(ps, (lambda mm: lambda lo, hi: h1T[:, mm, lo:hi])(m),
                          m, "half")

            # ---------------- L2 (2-bank pool forces m-serial) ------------
            h2T = apool.tile([128, M2, BC], FP8, tag="h2T")
            for m in range(M2):
                ps = psmm.tile([128, BC], F32, tag="mm", name=f"l2_{m}")
                tower_tile(ps, w2_sb, m, K2T, h1T)
                conv_relu(ps, (lambda mm: lambda lo, hi: h2T[:, mm, lo:hi])(m),
                          M1 + m,
                          L2CONV if L2CONV not in ("alt", "rev")
                          else ("act" if (m % 2 == 0) == (L2CONV == "alt")
                                else "dve"))

            # ---------------- L3 (chunk-aligned conv quarters) ------------
            h3T = apool.tile([128, M3, BC], FP8, tag="h3T")
            for m in range(M3):
                ps = psmm.tile([128, BC], F32, tag="mm", name=f"l3_{m}")
                tower_tile(ps, w3_sb, m, K3T, h2T)
                h3cv = os.environ.get("K_H3CONV", "whole")
                if zb and h3cv == "quart":
                    for ci in range(4):
                        lo, hi = 128 * ci, 128 * ci + 128
                        eng = nc.scalar if ci < 2 else nc.vector
                        if ci < 2:
                            nc.scalar.activation(
                                out=h3T[:, m, lo:hi], in_=ps[:, lo:hi],
                                func=AF.Relu, scale=INV,
                            )
                        else:
                            nc.vector.tensor_scalar(
                                out=h3T[:, m, lo:hi], in0=ps[:, lo:hi],
                                scalar1=INV, scalar2=0.0,
                                op0=ALU.mult, op1=ALU.max,
                            )
                elif zb and h3cv == "whole":
                    conv_relu(
                        ps,
                        (lambda mm: lambda lo, hi: h3T[:, mm, lo:hi])(m),
                        M1 + M2 + m, "act" if m == 0 else "dve")
                elif zb and h3cv == "wrev":
                    conv_relu(
                        ps,
                        (lambda mm: lambda lo, hi: h3T[:, mm, lo:hi])(m),
                        M1 + M2 + m, "dve" if m == 0 else "act")
                else:
                    conv_relu(
                        ps,
                        (lambda mm: lambda lo, hi: h3T[:, mm, lo:hi])(m),
                        M1 + M2 + m, "half")

            # ---------------- head: lg with examples on partitions --------
            # mz (ready early) is pre-written into the head psum scaled by
            # 4096; head matmuls accumulate on top (start=False throughout),
            # so sigmoid reads the finished logit*4096 straight from PSUM.
            MZPS = os.environ.get("K_MZPS", "1") == "1"
            lg_ps = pslg.tile([128, 4], F32, tag="lg", name="lgps")
            if MZPS:
                # A start=True dummy matmul (zero operands) resets the bank's
                # accumulation state so the DVE pre-write below is not wiped
                # by stale zero-on-write bits when the start=False head
                # matmuls land on real hardware.
                nc.tensor.matmul(
                    out=lg_ps[:], lhsT=x0_sb[:, 13, 0:128],
                    rhs=x0_sb[:, 13, 0:4], start=True, stop=True,
                    skip_group_check=True,
                )
                nc.vector.tensor_scalar(
                    out=lg_ps[:], in0=mz[:], scalar1=float(S * S),
                    scalar2=None, op0=ALU.mult,
                )
            for kt in range(KHT):
                for c in range(4):
                    nc.tensor.matmul(
                        out=lg_ps[:, c:c + 1],
                        lhsT=h3T[:, kt, 128 * c:128 * c + 128],
                        rhs=ws_sb[:, 14, kt:kt + 1],
                        start=(not MZPS and kt == 0 and c == 0),
                        stop=(kt == KHT - 1 and c == 3),
                        skip_group_check=True,
                    )

            # ---------------- sigmoid + store -----------------------------
            res_ap = res[:, 0, :, 0]
            if zb:
                nc.scalar.activation(out=res_ap, in_=lg_ps[:],
                                     func=AF.Sigmoid, scale=INV2)
            else:
                nc.scalar.activation(
                    out=res_ap, in_=lg_ps[:], func=AF.Sigmoid, scale=INV2,
                    bias=csc[:, 2:3],
                )
            if KVOUT:
                nc.gpsimd.trigger_dma(count=None)
            else:
                nc.sync.dma_start(a["outf"][:], res_ap)

        if R == 1:
            _body()
        else:
            with tc.For_i(0, R, 1):
                _body()


def build_program(zb):
    key = ("nc", zb, os.environ.get("K_REPEAT", "1"))
    if key in _CACHE:
        return _CACHE[key]
    nc = bacc.Bacc("TRN2", target_bir_lowering=False, debug=False, num_devices=NC)
    aps = {}

    def din(name, shape, dt):
        aps[name] = nc.dram_tensor(name, shape, dt, kind="ExternalInput").ap()

    din("x0T", [128, KT, BC], FP8)
    din("w1x", [128, M1, KTP, 128], FP8)
    din("w2x", [128, M2, K2T, 128], FP8)
    din("w3x", [128, M3, K3T, 128], FP8)
    din("wsx", [128, 15, 4], FP8)
    if not zb:
        din("cbx", [128, KT, 3], FP8)
        din("bx", [128, 16], F32)
    if os.environ.get("K_KVOUT", "0") == "1":
        aps["out"] = nc.dram_tensor(
            "out", [4, 128, 1, 1], F32, kind="ExternalOutput").ap()
    else:
        aps["outf"] = nc.dram_tensor(
            "outf", [128, 4], F32, kind="ExternalOutput").ap()
    _CACHE["aps"] = aps

    with tile.TileContext(nc) as tc:
        _emit(tc, zb)
    nc.compile()
    _CACHE[key] = nc
    return nc


def prepare_in_maps(inputs):
    import ml_dtypes
    f8 = ml_dtypes.float8_e4m3fn

    ids = np.asarray(inputs["ids"]).astype(np.int64)
    emb = np.asarray(inputs["emb"], dtype=np.float32)
    cross_w = np.asarray(inputs["cross_w"], dtype=np.float32)
    cross_b = np.asarray(inputs["cross_b"], dtype=np.float32)
    w1 = np.asarray(inputs["w1"], dtype=np.float32)
    w2 = np.asarray(inputs["w2"], dtype=np.float32)
    w3 = np.asarray(inputs["w3"], dtype=np.float32)
    b1 = np.asarray(inputs["b1"], dtype=np.float32)
    b2 = np.asarray(inputs["b2"], dtype=np.float32)
    b3 = np.asarray(inputs["b3"], dtype=np.float32)
    out_w = np.asarray(inputs["out_w"], dtype=np.float32)
    out_b = np.float32(np.asarray(inputs["out_b"], dtype=np.float32))

    zb = not (
        np.any(cross_b) or np.any(b1) or np.any(b2) or np.any(b3)
        or float(out_b) != 0.0
    )

    def lhst(w, ktiles, mtiles, kpad=0):
        # [K, M] -> [128, mtiles, ktiles+kpad, 128], scaled fp8
        arr = (w * S).astype(f8).reshape(ktiles, 128, mtiles, 128) \
            .transpose(1, 2, 0, 3)
        if kpad:
            z = np.zeros((128, mtiles, kpad, 128), f8)
            arr = np.concatenate([arr, z], axis=2)
        return np.ascontiguousarray(arr)

    w4 = np.concatenate([cross_w.T, out_w[:D].reshape(D, 1)], axis=1)  # [D,4]
    wsx = np.zeros((128, 15, 4), f8)
    wsx[:, 0:KT, :] = (w4 * S).astype(f8).reshape(KT, 128, 4).transpose(1, 0, 2)
    wsx[:, 14, 0:KHT] = (out_w[D:, 0] * S).astype(f8).reshape(KHT, 128).T
    shared = dict(
        w1x=lhst(w1, KT, M1, kpad=1),
        w2x=lhst(w2, K2T, M2),
        w3x=lhst(w3, K3T, M3),
        wsx=np.ascontiguousarray(wsx),
    )
    if not zb:
        shared["cbx"] = np.ascontiguousarray(
            (cross_b.T * S).astype(f8).reshape(KT, 128, 3).transpose(1, 0, 2)
        )
        bx = np.zeros((128, 16), np.float32)
        bx[:, 0:M1] = (b1 * S).reshape(M1, 128).T
        bx[:, M1:M1 + M2] = (b2 * S).reshape(M2, 128).T
        bx[:, M1 + M2:M1 + M2 + M3] = (b3 * S).reshape(M3, 128).T
        bx[0, 14] = out_b
        shared["bx"] = bx

    emb8 = (emb * S).astype(f8)
    in_maps = []
    for c in range(NC):
        idsc = ids[c * BC:(c + 1) * BC].reshape(-1)
        x0c = emb8[idsc].reshape(BC, D)                   # [512, 1664] fp8
        x0T = np.ascontiguousarray(
            x0c.reshape(BC, KT, 128).transpose(2, 1, 0)   # [128, 13, 512]
        )
        in_maps.append(dict(x0T=x0T, **shared))
    return in_maps, zb


def kernel(**inputs):
    in_maps, zb = prepare_in_maps(inputs)
    nc = build_program(zb)
    res = bass_utils.run_bass_kernel_spmd(nc, in_maps, core_ids=list(range(NC)))
    key = "out" if os.environ.get("K_KVOUT", "0") == "1" else "outf"
    out = np.concatenate(
        [res.results[c][key].reshape(4, 128) if key == "out"
         else res.results[c][key].T.reshape(BC)
         for c in range(NC)], axis=None,
    ).reshape(B, 1)
    return out.astype(np.float32)
